# revision 15
# baseline (speedup 1.0000x reference)
"""GAT (3-layer) Trainium2 kernel — 8-core SPMD.

Sharding: edges partitioned by destination node (sorted by dst); each core owns a
contiguous node range (aggregation fully local). Per layer: sharded node matmuls
build a [h | a_src | a_dst] table shard, AllGather replicates the table, then the
edge phase gathers source rows per 128-edge chunk and aggregates messages per
128-node tile via one-hot matmuls accumulated in PSUM.
"""

import numpy as np
import ml_dtypes

import concourse.bass as bass_mod
import concourse.bacc as bacc
import concourse.mybir as mybir
from concourse.tile import TileContext
from concourse.bass_utils import run_bass_kernel_spmd

# problem constants
IN = 256
HID = 32
OUT = 32
H = 8
ED = 32
NEG = 0.2
EPS = 1e-16
NC = 8

ROW = 384                    # bf16 units per table row: h[256]|asrc f32(16)|adst f32(16)|pad(96)
FDIM = 272                   # fp32 columns computed per node (h 256 + asrc 8 + adst 8)

BF16 = mybir.dt.bfloat16
F32 = mybir.dt.float32
I16 = mybir.dt.int16
NPBF16 = ml_dtypes.bfloat16

NLAYERS = 3
DEBUG_DUMP = False

# graph-size-derived globals (set by _configure)
N = E = NS_REAL = NT = NS = NPAD = NHALF = 0


def _configure(n, e):
    global N, E, NS_REAL, NT, NS, NPAD, NHALF
    N, E = n, e
    assert n % NC == 0
    NS_REAL = n // NC
    NT = (NS_REAL + 127) // 128
    NS = NT * 128
    NPAD = NC * NS
    NHALF = NPAD // 2


# ----------------------------------------------------------------------------
# host-side preprocessing
# ----------------------------------------------------------------------------

def _fold_params(params):
    out = []
    for p in params:
        W, att_src, att_dst, We, att_edge, bias, skip_W, skip_b = [
            np.asarray(t, np.float32) for t in p]
        c = W.shape[1] // H
        A_src = np.zeros((H * c, H), np.float32)
        A_dst = np.zeros((H * c, H), np.float32)
        for h in range(H):
            A_src[h * c:(h + 1) * c, h] = att_src[h]
            A_dst[h * c:(h + 1) * c, h] = att_dst[h]
        Wfold = np.concatenate([W, W @ A_src, W @ A_dst], axis=1)  # [256, 272]
        W_eff = np.einsum('khc,hc->kh', We.reshape(ED, H, c), att_edge)  # [32, 8]
        out.append(dict(Wfold=Wfold, W_eff=W_eff, skip_W=skip_W,
                        brow=(bias + skip_b).astype(np.float32),
                        out_dim=skip_W.shape[1]))
    return out


def _preprocess(edge_index, edge_attr, folded):
    src = np.asarray(edge_index[0], np.int64)
    dst = np.asarray(edge_index[1], np.int64)
    order = np.argsort(dst, kind='stable')
    src = src[order]
    dst = dst[order]
    ea = np.asarray(edge_attr, np.float32)[order]
    aeL = np.stack([ea @ f['W_eff'] for f in folded], axis=1)  # [E, 3, 8] f32

    src_pad = (src // NS_REAL) * NS + (src % NS_REAL)
    core_of = dst // NS_REAL
    loc = dst - core_of * NS_REAL
    tile_of = loc // 128

    edge_lists = [[None] * NT for _ in range(NC)]
    cnt = np.zeros((NC, NT), np.int64)
    for k in range(NC):
        lo = np.searchsorted(dst, k * NS_REAL, 'left')
        hi = np.searchsorted(dst, (k + 1) * NS_REAL, 'left')
        t_arr = tile_of[lo:hi]
        for t in range(NT):
            idx = np.nonzero(t_arr == t)[0] + lo
            edge_lists[k][t] = idx
            cnt[k, t] = len(idx)

    CH = np.maximum(1, -(-cnt.max(axis=0) // 128))  # [NT] chunks per tile
    SUMC = int(CH.sum())
    nslot = SUMC * 128

    per_core = []
    for k in range(NC):
        gidx = np.zeros(nslot, np.int32)   # pad slots gather row 0
        dcol = np.zeros(nslot, np.float32)
        dloc = np.zeros(nslot, np.int32)
        aep = np.full((3, nslot, H), -1e30, np.float32)
        for t in range(NT):
            base = int(CH[:t].sum()) * 128
            eidx = edge_lists[k][t]
            n = len(eidx)
            gidx[base:base + n] = src_pad[eidx].astype(np.int32)
            dcol[base:base + n] = (loc[eidx] - t * 128).astype(np.float32)
            dloc[base:base + n] = loc[eidx].astype(np.int32)
            aep[:, base:base + n, :] = np.transpose(aeL[eidx], (1, 0, 2))
        per_core.append(dict(gidx=gidx, dcol=dcol, dloc=dloc, aep=aep))

    return per_core, dict(CH=CH, SUMC=SUMC)


def _pack_inputs(x, folded, per_core, sched):
    CH, SUMC = sched['CH'], sched['SUMC']
    x = np.asarray(x, np.float32)

    # [128, 3*2*FDIM]: Wfold per layer l, k-chunk kk at cols (l*2+kk)*FDIM
    wf = np.zeros((128, 3 * 2 * FDIM), np.float32)
    sk = np.zeros((128, 3 * 2 * 256), np.float32)
    br = np.zeros((128, 3 * 256), np.float32)
    for l, f in enumerate(folded):
        W = f['Wfold'].reshape(2, 128, FDIM)
        for kk in range(2):
            wf[:, (l * 2 + kk) * FDIM:(l * 2 + kk + 1) * FDIM] = W[kk]
        Ws = f['skip_W'].reshape(2, 128, f['out_dim'])
        for kk in range(2):
            sk[:, (l * 2 + kk) * 256:(l * 2 + kk) * 256 + f['out_dim']] = Ws[kk]
        br[:, l * 256:l * 256 + f['out_dim']] = f['brow'][None, :]

    in_maps = []
    for k in range(NC):
        pc = per_core[k]
        xs = np.zeros((NS, IN), np.float32)
        xs[:NS_REAL] = x[k * NS_REAL:(k + 1) * NS_REAL]
        # xT layout: [128 part, (k ns)] with xT[p, kk*NS + j] = x[j, kk*128+p]
        xT0 = np.zeros((128, 2 * NS), np.float32)
        xt_full = xs.T  # [256, NS]
        xT0[:, 0:NS] = xt_full[0:128]
        xT0[:, NS:2 * NS] = xt_full[128:256]

        gidx_w = np.ascontiguousarray(pc['gidx'].reshape(SUMC, 128).T).astype(np.int32)
        d = pc['dcol'].reshape(SUMC, 128)
        dcol = np.ascontiguousarray(d.T).astype(NPBF16)
        dloc = np.ascontiguousarray(pc['dloc'].reshape(SUMC, 128).T).astype(np.int32)
        aep = np.ascontiguousarray(
            pc['aep'].reshape(3, SUMC, 128, H).transpose(0, 2, 1, 3).reshape(3, 128, SUMC * H)
        ).astype(np.float32)
        in_maps.append({
            'xT0': xT0, 'wf': wf, 'sk': sk, 'br': br,
            'gidx': gidx_w, 'dcol': dcol, 'dloc': dloc, 'aep': aep,
        })
    return in_maps


# ----------------------------------------------------------------------------
# device program
# ----------------------------------------------------------------------------

def build_program(sched, collective=True, ndev=None):
    CH, SUMC = sched['CH'], sched['SUMC']

    nc = bacc.Bacc("TRN2", num_devices=(NC if ndev is None else ndev))

    xT0_d = nc.dram_tensor('xT0', [128, 2 * NS], F32, kind='ExternalInput')
    wf_d = nc.dram_tensor('wf', [128, 3 * 2 * FDIM], F32, kind='ExternalInput')
    sk_d = nc.dram_tensor('sk', [128, 3 * 2 * 256], F32, kind='ExternalInput')
    br_d = nc.dram_tensor('br', [128, 3 * 256], F32, kind='ExternalInput')
    gidx_d = nc.dram_tensor('gidx', [128, SUMC], mybir.dt.int32, kind='ExternalInput')
    dcol_d = nc.dram_tensor('dcol', [128, SUMC], BF16, kind='ExternalInput')
    dloc_d = nc.dram_tensor('dloc', [128, SUMC], mybir.dt.int32, kind='ExternalInput')
    aep_d = nc.dram_tensor('aep', [3, 128, SUMC * H], F32, kind='ExternalInput')
    out_d = nc.dram_tensor('out', [NS, OUT], F32, kind='ExternalOutput')
    dbg = {}
    if DEBUG_DUMP:
        ch0 = int(CH[0])
        dbg['hg'] = nc.dram_tensor('dbg_hg', [128, ch0 * ROW], F32, kind='ExternalOutput')
        dbg['asd'] = nc.dram_tensor('dbg_asd', [128, ch0 * 64], F32, kind='ExternalOutput')
        dbg['logit'] = nc.dram_tensor('dbg_logit', [128, ch0 * 8], F32, kind='ExternalOutput')
        dbg['mext'] = nc.dram_tensor('dbg_mext', [128, ch0 * 264], BF16, kind='ExternalOutput')
        dbg['S'] = nc.dram_tensor('dbg_S', [128, ch0 * 128], BF16, kind='ExternalOutput')
        dbg['agg'] = nc.dram_tensor('dbg_agg', [128, 264], F32, kind='ExternalOutput')

    zeros48_d = nc.inline_tensor(np.zeros((128, 96), NPBF16), name='zeros48')
    iota_rep_d = nc.inline_tensor(
        np.tile(np.arange(128, dtype=NPBF16), (128, 1)), name='iota_rep')
    ident_d = nc.inline_tensor(np.eye(128, dtype=np.float32), name='ident')

    with TileContext(nc) as tc:
        with (
            tc.tile_pool(name='dram', bufs=1, space='DRAM') as dpool,
            tc.tile_pool(name='const', bufs=1) as cpool,
            tc.tile_pool(name='xt', bufs=2) as xtpool,
            tc.tile_pool(name='work', bufs=2) as wpool,
            tc.tile_pool(name='gat', bufs=2) as gpool,
            tc.tile_pool(name='small', bufs=3) as spool,
            tc.tile_pool(name='ph', bufs=2, space='PSUM') as ppool_h,
            tc.tile_pool(name='pe', bufs=2, space='PSUM') as ppool_e,
            tc.tile_pool(name='pt', bufs=2, space='PSUM') as ppool_t,
        ):
            tshard = dpool.tile([NS, ROW], BF16)
            table = dpool.tile([NPAD, ROW], BF16)
            xt_a = dpool.tile([128, 2 * NS], F32)
            xt_b = dpool.tile([128, 2 * NS], F32)

            wf_sb = cpool.tile([128, 3 * 2 * FDIM], F32, tag='wf')
            nc.sync.dma_start(wf_sb[:], wf_d[:, :])
            sk_sb = cpool.tile([128, 3 * 2 * 256], F32, tag='sk')
            nc.sync.dma_start(sk_sb[:], sk_d[:, :])
            br_sb = cpool.tile([128, 3 * 256], F32, tag='br')
            nc.sync.dma_start(br_sb[:], br_d[:, :])
            irep = cpool.tile([128, 128], BF16, tag='irep')
            nc.sync.dma_start(irep[:], iota_rep_d[:, :])
            z48 = cpool.tile([128, 96], BF16, tag='z48')
            nc.sync.dma_start(z48[:], zeros48_d[:, :])
            ident = cpool.tile([128, 128], F32, tag='ident')
            nc.sync.dma_start(ident[:], ident_d[:, :])

            for l in range(NLAYERS):
                xt_src = [xT0_d, xt_a, xt_b][l]
                xt_dst = [xt_a, xt_b, None][l]
                last = (l == NLAYERS - 1)

                xt_sb = xtpool.tile([128, 2 * NS], F32, tag='xt')
                nc.sync.dma_start(xt_sb[:], xt_src[:, :])

                # ---- node phase ----
                for t in range(NT):
                    ph = ppool_h.tile([128, FDIM], F32, tag='ph')
                    for kk in range(2):
                        nc.tensor.matmul(
                            ph[:],
                            lhsT=xt_sb[:, kk * NS + t * 128: kk * NS + (t + 1) * 128],
                            rhs=wf_sb[:, (l * 2 + kk) * FDIM:(l * 2 + kk + 1) * FDIM],
                            start=(kk == 0), stop=(kk == 1),
                        )
                    hsb = wpool.tile([128, 256], BF16, tag='hsb')
                    nc.vector.tensor_copy(hsb[:], ph[:, 0:256])
                    nc.sync.dma_start(tshard[t * 128:(t + 1) * 128, 0:256], hsb[:])
                    asb = wpool.tile([128, 16], F32, tag='asb')
                    nc.vector.tensor_copy(asb[:], ph[:, 256:272])
                    nc.sync.dma_start(
                        tshard[t * 128:(t + 1) * 128, 256:288].bitcast(F32), asb[:])
                    if l == 0:
                        nc.sync.dma_start(tshard[t * 128:(t + 1) * 128, 288:ROW], z48[:])

                # ---- replicate table ----
                if collective:
                    nc.gpsimd.collective_compute(
                        'AllGather', mybir.AluOpType.bypass,
                        replica_groups=[list(range(NC))],
                        ins=[tshard[:, :]], outs=[table[:, :]],
                    )
                else:
                    for t in range(NT):
                        cp = wpool.tile([128, ROW], BF16, tag='cp')
                        nc.sync.dma_start(cp[:], tshard[t * 128:(t + 1) * 128, :])
                        nc.sync.dma_start(table[t * 128:(t + 1) * 128, :], cp[:])

                # ---- edge phase ----
                for t in range(NT):
                    ch = int(CH[t])
                    coff = int(CH[:t].sum())

                    gsb = spool.tile([128, ch], mybir.dt.int32, tag='gsb')
                    nc.sync.dma_start(gsb[:], gidx_d[:, coff:coff + ch])
                    hg = gpool.tile([128, ch * ROW], BF16, tag='hg')
                    for c in range(ch):
                        nc.gpsimd.indirect_dma_start(
                            out=hg[:, c * ROW:(c + 1) * ROW], out_offset=None,
                            in_=table[:, :],
                            in_offset=bass_mod.IndirectOffsetOnAxis(
                                ap=gsb[:, c:c + 1], axis=0),
                        )

                    dc = spool.tile([128, ch], BF16, tag='dc')
                    nc.sync.dma_start(dc[:], dcol_d[:, coff:coff + ch])
                    S = wpool.tile([128, ch * 128], BF16, tag='S')
                    nc.vector.tensor_tensor(
                        S[:].rearrange('p (c n) -> p c n', n=128),
                        dc[:].unsqueeze(2).broadcast_to([128, ch, 128]),
                        irep[:].unsqueeze(1).broadcast_to([128, ch, 128]),
                        mybir.AluOpType.is_equal,
                    )
                    # gather [a_src | a_dst] rows of the DESTINATION nodes (local shard)
                    dlsb = spool.tile([128, ch], mybir.dt.int32, tag='dlsb')
                    nc.sync.dma_start(dlsb[:], dloc_d[:, coff:coff + ch])
                    asd = spool.tile([128, ch * 32], BF16, tag='asd')
                    for c in range(ch):
                        nc.gpsimd.indirect_dma_start(
                            out=asd[:, c * 32:(c + 1) * 32], out_offset=None,
                            in_=tshard[:, :], element_offset=256,
                            in_offset=bass_mod.IndirectOffsetOnAxis(
                                ap=dlsb[:, c:c + 1], axis=0),
                        )

                    aept = spool.tile([128, ch * 8], F32, tag='aept')
                    nc.sync.dma_start(aept[:], aep_d[l, :, coff * 8:(coff + ch) * 8])
                    logit = spool.tile([128, ch * 8], F32, tag='logit')
                    nc.vector.tensor_tensor(
                        logit[:].rearrange('p (c h) -> p c h', h=8),
                        aept[:].rearrange('p (c h) -> p c h', h=8),
                        asd[:].bitcast(F32).rearrange('p (c f) -> p c f', f=16)[:, :, 8:16],
                        mybir.AluOpType.add)
                    nc.vector.tensor_tensor(
                        logit[:].rearrange('p (c h) -> p c h', h=8),
                        logit[:].rearrange('p (c h) -> p c h', h=8),
                        hg[:].bitcast(F32).rearrange('p (c f) -> p c f', f=ROW // 2)[:, :, 128:136],
                        mybir.AluOpType.add,
                    )
                    nc.vector.scalar_tensor_tensor(
                        logit[:], logit[:], NEG, logit[:],
                        mybir.AluOpType.mult, mybir.AluOpType.max,
                    )
                    mext = gpool.tile([128, ch * 264], BF16, tag='mext')
                    mview = mext[:].rearrange('p (c f) -> p c f', f=264)
                    nc.scalar.activation(
                        mview[:, :, 256:264],
                        logit[:].rearrange('p (c h) -> p c h', h=8),
                        mybir.ActivationFunctionType.Exp,
                    )
                    nc.vector.tensor_tensor(
                        mview[:, :, 0:256].rearrange('p c (h f) -> p c h f', f=32),
                        hg[:].rearrange('p (c f) -> p c f', f=ROW)[:, :, 0:256]
                            .rearrange('p c (h f) -> p c h f', f=32),
                        mview[:, :, 256:264].unsqueeze(3).broadcast_to([128, ch, 8, 32]),
                        mybir.AluOpType.mult,
                    )

                    pagg = ppool_e.tile([128, 264], F32, tag='pagg')
                    for c in range(ch):
                        nc.tensor.matmul(
                            pagg[:],
                            lhsT=S[:, c * 128:(c + 1) * 128],
                            rhs=mext[:, c * 264:(c + 1) * 264],
                            start=(c == 0), stop=(c == ch - 1),
                        )

                    psk = ppool_e.tile([128, 256], F32, tag='psk')
                    for kk in range(2):
                        nc.tensor.matmul(
                            psk[:],
                            lhsT=xt_sb[:, kk * NS + t * 128: kk * NS + (t + 1) * 128],
                            rhs=sk_sb[:, (l * 2 + kk) * 256:(l * 2 + kk + 1) * 256],
                            start=(kk == 0), stop=(kk == 1),
                        )

                    if DEBUG_DUMP and l == 0 and t == 0:
                        nc.sync.dma_start(dbg['hg'][:, :], hg[:])
                        nc.sync.dma_start(dbg['asd'][:, :], asd[:])
                        nc.sync.dma_start(dbg['logit'][:, :], logit[:])
                        nc.sync.dma_start(dbg['mext'][:, :], mext[:])
                        nc.sync.dma_start(dbg['S'][:, :], S[:])
                        aggc = wpool.tile([128, 264], F32, tag='aggc')
                        nc.vector.tensor_copy(aggc[:], pagg[:])
                        nc.sync.dma_start(dbg['agg'][:, :], aggc[:])
                    den = spool.tile([128, 8], F32, tag='den')
                    nc.vector.tensor_scalar_add(den[:], pagg[:, 256:264], EPS)
                    rec = spool.tile([128, 8], F32, tag='rec')
                    nc.vector.reciprocal(rec[:], den[:])
                    normed = wpool.tile([128, 256], F32, tag='normed')
                    nc.vector.tensor_tensor(
                        normed[:].rearrange('p (h f) -> p h f', f=32),
                        pagg[:, 0:256].rearrange('p (h f) -> p h f', f=32),
                        rec[:].unsqueeze(2).broadcast_to([128, 8, 32]),
                        mybir.AluOpType.mult,
                    )

                    if not last:
                        z = wpool.tile([128, 256], F32, tag='z')
                        nc.vector.tensor_tensor(z[:], normed[:], psk[:], mybir.AluOpType.add)
                        nc.vector.tensor_tensor(
                            z[:], z[:],
                            br_sb[:, l * 256:(l + 1) * 256],
                            mybir.AluOpType.add,
                        )
                        zm = wpool.tile([128, 256], F32, tag='zm')
                        nc.vector.tensor_scalar_min(zm[:], z[:], 0.0)
                        ez = wpool.tile([128, 256], F32, tag='ez')
                        nc.scalar.activation(ez[:], zm[:], mybir.ActivationFunctionType.Exp)
                        xn = wpool.tile([128, 256], F32, tag='xn')
                        nc.vector.scalar_tensor_tensor(
                            xn[:], ez[:], -1.0, z[:],
                            mybir.AluOpType.add, mybir.AluOpType.max,
                        )
                        for kk in range(2):
                            pT = ppool_t.tile([128, 128], F32, tag='ptmp')
                            nc.tensor.transpose(pT[:], xn[:, kk * 128:(kk + 1) * 128], ident[:])
                            blk = wpool.tile([128, 128], F32, tag='blk')
                            nc.vector.tensor_copy(blk[:], pT[:])
                            nc.sync.dma_start(xt_dst[:, kk * NS + t * 128: kk * NS + (t + 1) * 128], blk[:])
                    else:
                        s1 = wpool.tile([128, 128], F32, tag='s1')
                        nc.vector.tensor_tensor(s1[:], normed[:, 0:128], normed[:, 128:256], mybir.AluOpType.add)
                        s2 = wpool.tile([128, 64], F32, tag='s2')
                        nc.vector.tensor_tensor(s2[:], s1[:, 0:64], s1[:, 64:128], mybir.AluOpType.add)
                        s3 = wpool.tile([128, 32], F32, tag='s3')
                        nc.vector.tensor_tensor(s3[:], s2[:, 0:32], s2[:, 32:64], mybir.AluOpType.add)
                        pre = wpool.tile([128, 32], F32, tag='pre')
                        nc.vector.tensor_tensor(
                            pre[:], psk[:, 0:32],
                            br_sb[:, 2 * 256:2 * 256 + 32],
                            mybir.AluOpType.add,
                        )
                        fin = wpool.tile([128, 32], F32, tag='fin')
                        nc.vector.scalar_tensor_tensor(
                            fin[:], s3[:], 1.0 / H, pre[:],
                            mybir.AluOpType.mult, mybir.AluOpType.add,
                        )
                        nc.sync.dma_start(out_d[t * 128:(t + 1) * 128, :], fin[:])

    nc.compile()
    return nc


# ----------------------------------------------------------------------------
# entry point
# ----------------------------------------------------------------------------

def kernel(x, edge_index, edge_attr, params):
    x = np.asarray(x)
    _configure(x.shape[0], np.asarray(edge_index).shape[1])
    folded = _fold_params(params)
    per_core, sched = _preprocess(edge_index, edge_attr, folded)
    in_maps = _pack_inputs(x, folded, per_core, sched)
    nc = build_program(sched)
    res = run_bass_kernel_spmd(nc, in_maps, core_ids=list(range(NC)))
    out = np.empty((N, OUT), np.float32)
    for k in range(NC):
        out[k * NS_REAL:(k + 1) * NS_REAL] = res.results[k]['out'][:NS_REAL]
    return out


# revision 20
# speedup vs baseline: 1.3198x; 1.3198x over previous
"""GAT (3-layer) Trainium2 kernel — 8-core SPMD.

Sharding: edges partitioned by destination node (sorted by dst); each core owns a
contiguous node range (aggregation fully local). Per layer: sharded node matmuls
build a [h | a_src | a_dst] table shard, AllGather replicates the table, then the
edge phase gathers source rows per 128-edge chunk and aggregates messages per
128-node tile via one-hot matmuls accumulated in PSUM.
"""

import numpy as np
import ml_dtypes

import concourse.bass as bass_mod
import concourse.bacc as bacc
import concourse.mybir as mybir
from concourse.tile import TileContext
from concourse.bass_utils import run_bass_kernel_spmd

# problem constants
IN = 256
HID = 32
OUT = 32
H = 8
ED = 32
NEG = 0.2
EPS = 1e-16
NC = 8

ROW = 288                    # bf16 units per table row: h[256] | asrc f32(16) | adst f32(16)
FDIM = 272                   # fp32 columns computed per node (h 256 + asrc 8 + adst 8)

BF16 = mybir.dt.bfloat16
F32 = mybir.dt.float32
I16 = mybir.dt.int16
NPBF16 = ml_dtypes.bfloat16

NLAYERS = 3
DEBUG_DUMP = False

# graph-size-derived globals (set by _configure)
N = E = NS_REAL = NT = NS = NPAD = NHALF = 0


def _configure(n, e):
    global N, E, NS_REAL, NT, NS, NPAD, NHALF
    N, E = n, e
    assert n % NC == 0
    NS_REAL = n // NC
    NT = (NS_REAL + 127) // 128
    NS = NT * 128
    NPAD = NC * NS
    NHALF = NPAD // 2


# ----------------------------------------------------------------------------
# host-side preprocessing
# ----------------------------------------------------------------------------

def _fold_params(params):
    out = []
    for p in params:
        W, att_src, att_dst, We, att_edge, bias, skip_W, skip_b = [
            np.asarray(t, np.float32) for t in p]
        c = W.shape[1] // H
        A_src = np.zeros((H * c, H), np.float32)
        A_dst = np.zeros((H * c, H), np.float32)
        for h in range(H):
            A_src[h * c:(h + 1) * c, h] = att_src[h]
            A_dst[h * c:(h + 1) * c, h] = att_dst[h]
        Wfold = np.concatenate([W, W @ A_src, W @ A_dst], axis=1)  # [256, 272]
        W_eff = np.einsum('khc,hc->kh', We.reshape(ED, H, c), att_edge)  # [32, 8]
        out.append(dict(Wfold=Wfold, W_eff=W_eff, skip_W=skip_W,
                        brow=(bias + skip_b).astype(np.float32),
                        out_dim=skip_W.shape[1]))
    return out


def _preprocess(edge_index, edge_attr, folded):
    src = np.asarray(edge_index[0], np.int64)
    dst = np.asarray(edge_index[1], np.int64)
    order = np.argsort(dst, kind='stable')
    src = src[order]
    dst = dst[order]
    ea = np.asarray(edge_attr, np.float32)[order]
    aeL = np.stack([ea @ f['W_eff'] for f in folded], axis=1)  # [E, 3, 8] f32

    src_pad = (src // NS_REAL) * NS + (src % NS_REAL)
    core_of = dst // NS_REAL
    loc = dst - core_of * NS_REAL
    tile_of = loc // 128

    edge_lists = [[None] * NT for _ in range(NC)]
    cnt = np.zeros((NC, NT), np.int64)
    for k in range(NC):
        lo = np.searchsorted(dst, k * NS_REAL, 'left')
        hi = np.searchsorted(dst, (k + 1) * NS_REAL, 'left')
        t_arr = tile_of[lo:hi]
        for t in range(NT):
            idx = np.nonzero(t_arr == t)[0] + lo
            edge_lists[k][t] = idx
            cnt[k, t] = len(idx)

    CH = np.maximum(1, -(-cnt.max(axis=0) // 128))  # [NT] chunks per tile
    SUMC = int(CH.sum())
    nslot = SUMC * 128

    per_core = []
    for k in range(NC):
        gidx = np.zeros(nslot, np.int32)   # pad slots gather row 0
        dcol = np.zeros(nslot, np.float32)
        dloc = np.zeros(nslot, np.int32)
        aep = np.full((3, nslot, H), -1e30, np.float32)
        for t in range(NT):
            base = int(CH[:t].sum()) * 128
            eidx = edge_lists[k][t]
            n = len(eidx)
            gidx[base:base + n] = src_pad[eidx].astype(np.int32)
            dcol[base:base + n] = (loc[eidx] - t * 128).astype(np.float32)
            dloc[base:base + n] = loc[eidx].astype(np.int32)
            aep[:, base:base + n, :] = np.transpose(aeL[eidx], (1, 0, 2))
        per_core.append(dict(gidx=gidx, dcol=dcol, dloc=dloc, aep=aep))

    return per_core, dict(CH=CH, SUMC=SUMC)


def _pack_inputs(x, folded, per_core, sched):
    CH, SUMC = sched['CH'], sched['SUMC']
    x = np.asarray(x, np.float32)

    # [128, 3*2*FDIM]: Wfold per layer l, k-chunk kk at cols (l*2+kk)*FDIM
    wf = np.zeros((128, 3 * 2 * FDIM), np.float32)
    sk = np.zeros((128, 3 * 2 * 256), np.float32)
    br = np.zeros((128, 3 * 256), np.float32)
    for l, f in enumerate(folded):
        W = f['Wfold'].reshape(2, 128, FDIM)
        for kk in range(2):
            wf[:, (l * 2 + kk) * FDIM:(l * 2 + kk + 1) * FDIM] = W[kk]
        Ws = f['skip_W'].reshape(2, 128, f['out_dim'])
        for kk in range(2):
            sk[:, (l * 2 + kk) * 256:(l * 2 + kk) * 256 + f['out_dim']] = Ws[kk]
        br[:, l * 256:l * 256 + f['out_dim']] = f['brow'][None, :]

    in_maps = []
    for k in range(NC):
        pc = per_core[k]
        xs = np.zeros((NS, IN), np.float32)
        xs[:NS_REAL] = x[k * NS_REAL:(k + 1) * NS_REAL]
        # xT layout: [128 part, (k ns)] with xT[p, kk*NS + j] = x[j, kk*128+p]
        xT0 = np.zeros((128, 2 * NS), np.float32)
        xt_full = xs.T  # [256, NS]
        xT0[:, 0:NS] = xt_full[0:128]
        xT0[:, NS:2 * NS] = xt_full[128:256]

        gidx_w = np.ascontiguousarray(pc['gidx'].reshape(SUMC, 128).T).astype(np.int32)
        d = pc['dcol'].reshape(SUMC, 128)
        dcol = np.ascontiguousarray(d.T).astype(NPBF16)
        aep = np.ascontiguousarray(
            pc['aep'].reshape(3, SUMC, 128, H).transpose(0, 2, 1, 3).reshape(3, 128, SUMC * H)
        ).astype(NPBF16)
        in_maps.append({
            'xT0': xT0, 'wf': wf, 'sk': sk, 'br': br,
            'gidx': gidx_w, 'dcol': dcol, 'aep': aep,
        })
    return in_maps


# ----------------------------------------------------------------------------
# device program
# ----------------------------------------------------------------------------

def build_program(sched, collective=True, ndev=None):
    CH, SUMC = sched['CH'], sched['SUMC']

    nc = bacc.Bacc("TRN2", num_devices=(NC if ndev is None else ndev))

    xT0_d = nc.dram_tensor('xT0', [128, 2 * NS], F32, kind='ExternalInput')
    wf_d = nc.dram_tensor('wf', [128, 3 * 2 * FDIM], F32, kind='ExternalInput')
    sk_d = nc.dram_tensor('sk', [128, 3 * 2 * 256], F32, kind='ExternalInput')
    br_d = nc.dram_tensor('br', [128, 3 * 256], F32, kind='ExternalInput')
    gidx_d = nc.dram_tensor('gidx', [128, SUMC], mybir.dt.int32, kind='ExternalInput')
    dcol_d = nc.dram_tensor('dcol', [128, SUMC], BF16, kind='ExternalInput')
    aep_d = nc.dram_tensor('aep', [3, 128, SUMC * H], BF16, kind='ExternalInput')
    out_d = nc.dram_tensor('out', [NS, OUT], F32, kind='ExternalOutput')
    dbg = {}
    if DEBUG_DUMP:
        ch0 = int(CH[0])
        dbg['hg'] = nc.dram_tensor('dbg_hg', [128, ch0 * ROW], F32, kind='ExternalOutput')
        dbg['asd'] = nc.dram_tensor('dbg_asd', [128, ch0 * 64], F32, kind='ExternalOutput')
        dbg['logit'] = nc.dram_tensor('dbg_logit', [128, ch0 * 8], F32, kind='ExternalOutput')
        dbg['mext'] = nc.dram_tensor('dbg_mext', [128, ch0 * 264], BF16, kind='ExternalOutput')
        dbg['S'] = nc.dram_tensor('dbg_S', [128, ch0 * 128], BF16, kind='ExternalOutput')
        dbg['agg'] = nc.dram_tensor('dbg_agg', [128, 264], F32, kind='ExternalOutput')

    iota_rep_d = nc.inline_tensor(
        np.tile(np.arange(128, dtype=NPBF16), (128, 1)), name='iota_rep')
    ident_d = nc.inline_tensor(np.eye(128, dtype=np.float32), name='ident')
    identb_d = nc.inline_tensor(np.eye(128, dtype=NPBF16), name='identb')

    with TileContext(nc) as tc:
        with (
            tc.tile_pool(name='dram', bufs=1, space='DRAM') as dpool,
            tc.tile_pool(name='const', bufs=1) as cpool,
            tc.tile_pool(name='xt', bufs=2) as xtpool,
            tc.tile_pool(name='work', bufs=2) as wpool,
            tc.tile_pool(name='gat', bufs=2) as gpool,
            tc.tile_pool(name='small', bufs=3) as spool,
            tc.tile_pool(name='ph', bufs=2, space='PSUM') as ppool_h,
            tc.tile_pool(name='pe', bufs=2, space='PSUM') as ppool_e,
            tc.tile_pool(name='pg', bufs=1, space='PSUM') as ppool_g,
            tc.tile_pool(name='ps', bufs=1, space='PSUM') as ppool_s,
            tc.tile_pool(name='pt', bufs=2, space='PSUM') as ppool_t,
        ):
            tshard = dpool.tile([NS, ROW], BF16)
            table = dpool.tile([NPAD, ROW], BF16)
            xt_a = dpool.tile([128, 2 * NS], F32)
            xt_b = dpool.tile([128, 2 * NS], F32)

            wf_sb = cpool.tile([128, 3 * 2 * FDIM], F32, tag='wf')
            nc.sync.dma_start(wf_sb[:], wf_d[:, :])
            sk_sb = cpool.tile([128, 3 * 2 * 256], F32, tag='sk')
            nc.sync.dma_start(sk_sb[:], sk_d[:, :])
            br_sb = cpool.tile([128, 3 * 256], F32, tag='br')
            nc.sync.dma_start(br_sb[:], br_d[:, :])
            irep = cpool.tile([128, 128], BF16, tag='irep')
            nc.sync.dma_start(irep[:], iota_rep_d[:, :])
            ident = cpool.tile([128, 128], F32, tag='ident')
            nc.sync.dma_start(ident[:], ident_d[:, :])
            identb = cpool.tile([128, 128], BF16, tag='identb')
            nc.sync.dma_start(identb[:], identb_d[:, :])

            for l in range(NLAYERS):
                xt_src = [xT0_d, xt_a, xt_b][l]
                xt_dst = [xt_a, xt_b, None][l]
                last = (l == NLAYERS - 1)

                xt_sb = xtpool.tile([128, 2 * NS], F32, tag='xt')
                nc.sync.dma_start(xt_sb[:], xt_src[:, :])

                # ---- node phase ----
                for t in range(NT):
                    ph = ppool_h.tile([128, FDIM], F32, tag='ph')
                    for kk in range(2):
                        nc.tensor.matmul(
                            ph[:],
                            lhsT=xt_sb[:, kk * NS + t * 128: kk * NS + (t + 1) * 128],
                            rhs=wf_sb[:, (l * 2 + kk) * FDIM:(l * 2 + kk + 1) * FDIM],
                            start=(kk == 0), stop=(kk == 1),
                        )
                    hsb = wpool.tile([128, 256], BF16, tag='hsb')
                    nc.vector.tensor_copy(hsb[:], ph[:, 0:256])
                    nc.sync.dma_start(tshard[t * 128:(t + 1) * 128, 0:256], hsb[:])
                    asb = wpool.tile([128, 16], F32, tag='asb')
                    nc.vector.tensor_copy(asb[:], ph[:, 256:272])
                    nc.sync.dma_start(
                        tshard[t * 128:(t + 1) * 128, 256:288].bitcast(F32), asb[:])

                # ---- replicate table ----
                if collective:
                    nc.gpsimd.collective_compute(
                        'AllGather', mybir.AluOpType.bypass,
                        replica_groups=[list(range(NC))],
                        ins=[tshard[:, :]], outs=[table[:, :]],
                    )
                else:
                    for t in range(NT):
                        cp = wpool.tile([128, ROW], BF16, tag='cp')
                        nc.sync.dma_start(cp[:], tshard[t * 128:(t + 1) * 128, :])
                        nc.sync.dma_start(table[t * 128:(t + 1) * 128, :], cp[:])

                # ---- edge phase ----
                for t in range(NT):
                    ch = int(CH[t])
                    coff = int(CH[:t].sum())

                    gsb = spool.tile([128, ch], mybir.dt.int32, tag='gsb')
                    nc.sync.dma_start(gsb[:], gidx_d[:, coff:coff + ch])
                    hg = gpool.tile([128, ch * ROW], BF16, tag='hg')
                    for c in range(ch):
                        nc.gpsimd.indirect_dma_start(
                            out=hg[:, c * ROW:(c + 1) * ROW], out_offset=None,
                            in_=table[:, :],
                            in_offset=bass_mod.IndirectOffsetOnAxis(
                                ap=gsb[:, c:c + 1], axis=0),
                        )

                    dc = spool.tile([128, ch], BF16, tag='dc')
                    nc.sync.dma_start(dc[:], dcol_d[:, coff:coff + ch])
                    S = wpool.tile([128, ch * 128], BF16, tag='S')
                    nc.vector.tensor_tensor(
                        S[:].rearrange('p (c n) -> p c n', n=128),
                        dc[:].unsqueeze(2).broadcast_to([128, ch, 128]),
                        irep[:].unsqueeze(1).broadcast_to([128, ch, 128]),
                        mybir.AluOpType.is_equal,
                    )
                    # a_dst per edge: transpose each S chunk on PE, matmul vs local a_dst
                    adf = spool.tile([128, 16], F32, tag='adf')
                    nc.sync.dma_start(
                        adf[:], tshard[t * 128:(t + 1) * 128, 256:288].bitcast(F32))
                    adb = spool.tile([128, 8], BF16, tag='adb')
                    nc.vector.tensor_copy(adb[:], adf[:, 8:16])
                    plg = ppool_g.tile([128, ch * 8], F32, tag='plg')
                    for c in range(ch):
                        pTr = ppool_t.tile([128, 128], BF16, tag='ptmp')
                        nc.tensor.transpose(pTr[:], S[:, c * 128:(c + 1) * 128], identb[:])
                        stb = spool.tile([128, 128], BF16, tag='stb')
                        nc.vector.tensor_copy(stb[:], pTr[:])
                        nc.tensor.matmul(
                            plg[:, c * 8:(c + 1) * 8], lhsT=stb[:], rhs=adb[:],
                            start=True, stop=True)

                    aept = spool.tile([128, ch * 8], BF16, tag='aept')
                    nc.sync.dma_start(aept[:], aep_d[l, :, coff * 8:(coff + ch) * 8])
                    logit = spool.tile([128, ch * 8], F32, tag='logit')
                    nc.vector.tensor_tensor(logit[:], plg[:], aept[:], mybir.AluOpType.add)
                    nc.vector.tensor_tensor(
                        logit[:].rearrange('p (c h) -> p c h', h=8),
                        logit[:].rearrange('p (c h) -> p c h', h=8),
                        hg[:].bitcast(F32).rearrange('p (c f) -> p c f', f=ROW // 2)[:, :, 128:136],
                        mybir.AluOpType.add,
                    )
                    nc.vector.scalar_tensor_tensor(
                        logit[:], logit[:], NEG, logit[:],
                        mybir.AluOpType.mult, mybir.AluOpType.max,
                    )
                    mext = gpool.tile([128, ch * 264], BF16, tag='mext')
                    mview = mext[:].rearrange('p (c f) -> p c f', f=264)
                    nc.scalar.activation(
                        mview[:, :, 256:264],
                        logit[:].rearrange('p (c h) -> p c h', h=8),
                        mybir.ActivationFunctionType.Exp,
                    )
                    nc.vector.tensor_tensor(
                        mview[:, :, 0:256].rearrange('p c (h f) -> p c h f', f=32),
                        hg[:].rearrange('p (c f) -> p c f', f=ROW)[:, :, 0:256]
                            .rearrange('p c (h f) -> p c h f', f=32),
                        mview[:, :, 256:264].unsqueeze(3).broadcast_to([128, ch, 8, 32]),
                        mybir.AluOpType.mult,
                    )

                    pagg = ppool_e.tile([128, 264], F32, tag='pagg')
                    for c in range(ch):
                        nc.tensor.matmul(
                            pagg[:],
                            lhsT=S[:, c * 128:(c + 1) * 128],
                            rhs=mext[:, c * 264:(c + 1) * 264],
                            start=(c == 0), stop=(c == ch - 1),
                        )

                    psk = ppool_s.tile([128, 256], F32, tag='psk')
                    for kk in range(2):
                        nc.tensor.matmul(
                            psk[:],
                            lhsT=xt_sb[:, kk * NS + t * 128: kk * NS + (t + 1) * 128],
                            rhs=sk_sb[:, (l * 2 + kk) * 256:(l * 2 + kk + 1) * 256],
                            start=(kk == 0), stop=(kk == 1),
                        )

                    if DEBUG_DUMP and l == 0 and t == 0:
                        nc.sync.dma_start(dbg['hg'][:, :], hg[:])
                        nc.sync.dma_start(dbg['asd'][:, :], asd[:])
                        nc.sync.dma_start(dbg['logit'][:, :], logit[:])
                        nc.sync.dma_start(dbg['mext'][:, :], mext[:])
                        nc.sync.dma_start(dbg['S'][:, :], S[:])
                        aggc = wpool.tile([128, 264], F32, tag='aggc')
                        nc.vector.tensor_copy(aggc[:], pagg[:])
                        nc.sync.dma_start(dbg['agg'][:, :], aggc[:])
                    den = spool.tile([128, 8], F32, tag='den')
                    nc.vector.tensor_scalar_add(den[:], pagg[:, 256:264], EPS)
                    rec = spool.tile([128, 8], F32, tag='rec')
                    nc.vector.reciprocal(rec[:], den[:])
                    normed = wpool.tile([128, 256], F32, tag='normed')
                    nc.vector.tensor_tensor(
                        normed[:].rearrange('p (h f) -> p h f', f=32),
                        pagg[:, 0:256].rearrange('p (h f) -> p h f', f=32),
                        rec[:].unsqueeze(2).broadcast_to([128, 8, 32]),
                        mybir.AluOpType.mult,
                    )

                    if not last:
                        z = wpool.tile([128, 256], F32, tag='z')
                        nc.vector.tensor_tensor(z[:], normed[:], psk[:], mybir.AluOpType.add)
                        nc.vector.tensor_tensor(
                            z[:], z[:],
                            br_sb[:, l * 256:(l + 1) * 256],
                            mybir.AluOpType.add,
                        )
                        zm = wpool.tile([128, 256], F32, tag='zm')
                        nc.vector.tensor_scalar_min(zm[:], z[:], 0.0)
                        ez = wpool.tile([128, 256], F32, tag='ez')
                        nc.scalar.activation(ez[:], zm[:], mybir.ActivationFunctionType.Exp)
                        xn = wpool.tile([128, 256], F32, tag='xn')
                        nc.vector.scalar_tensor_tensor(
                            xn[:], ez[:], -1.0, z[:],
                            mybir.AluOpType.add, mybir.AluOpType.max,
                        )
                        for kk in range(2):
                            pT = ppool_t.tile([128, 128], F32, tag='ptmp')
                            nc.tensor.transpose(pT[:], xn[:, kk * 128:(kk + 1) * 128], ident[:])
                            blk = wpool.tile([128, 128], F32, tag='blk')
                            nc.vector.tensor_copy(blk[:], pT[:])
                            nc.sync.dma_start(xt_dst[:, kk * NS + t * 128: kk * NS + (t + 1) * 128], blk[:])
                    else:
                        s1 = wpool.tile([128, 128], F32, tag='s1')
                        nc.vector.tensor_tensor(s1[:], normed[:, 0:128], normed[:, 128:256], mybir.AluOpType.add)
                        s2 = wpool.tile([128, 64], F32, tag='s2')
                        nc.vector.tensor_tensor(s2[:], s1[:, 0:64], s1[:, 64:128], mybir.AluOpType.add)
                        s3 = wpool.tile([128, 32], F32, tag='s3')
                        nc.vector.tensor_tensor(s3[:], s2[:, 0:32], s2[:, 32:64], mybir.AluOpType.add)
                        pre = wpool.tile([128, 32], F32, tag='pre')
                        nc.vector.tensor_tensor(
                            pre[:], psk[:, 0:32],
                            br_sb[:, 2 * 256:2 * 256 + 32],
                            mybir.AluOpType.add,
                        )
                        fin = wpool.tile([128, 32], F32, tag='fin')
                        nc.vector.scalar_tensor_tensor(
                            fin[:], s3[:], 1.0 / H, pre[:],
                            mybir.AluOpType.mult, mybir.AluOpType.add,
                        )
                        nc.sync.dma_start(out_d[t * 128:(t + 1) * 128, :], fin[:])

    nc.compile()
    return nc


# ----------------------------------------------------------------------------
# entry point
# ----------------------------------------------------------------------------

def kernel(x, edge_index, edge_attr, params):
    x = np.asarray(x)
    _configure(x.shape[0], np.asarray(edge_index).shape[1])
    folded = _fold_params(params)
    per_core, sched = _preprocess(edge_index, edge_attr, folded)
    in_maps = _pack_inputs(x, folded, per_core, sched)
    nc = build_program(sched)
    res = run_bass_kernel_spmd(nc, in_maps, core_ids=list(range(NC)))
    out = np.empty((N, OUT), np.float32)
    for k in range(NC):
        out[k * NS_REAL:(k + 1) * NS_REAL] = res.results[k]['out'][:NS_REAL]
    return out
